# revision 11
# baseline (speedup 1.0000x reference)
"""Bass/Trainium2 kernel for batched int8 matmul with fp32 dequant epilogue.

Computes out[b, m, n] = alpha * sum_k a[b, m, k] * b[b, n, k] for
a, b int8 [256, 512, 128], out fp32 [256, 512, 512].

Strategy:
  - Shard the batch dim B=256 across 8 NeuronCores (32 batches/core).
  - int8 values convert EXACTLY to bf16 (8-bit significand covers +-256);
    products are ints <= 2^14 and the K=128 accumulation stays <= 2^21,
    exactly representable in the fp32 PSUM accumulator -> the bf16 matmul
    reproduces the int32-accumulated reference bit-exactly.
  - Host pre-transposes both operands to [B, K, M/N] so K lands on the
    SBUF partition dim (the PE contracts over partitions) with fully
    contiguous DMA rows.
  - K=128 means each [128m x 512n] output tile is a single matmul
    (no accumulation loop). alpha is folded into the PSUM->SBUF copy,
    alternating ScalarE/VectorE; fp32 out DMAs back to HBM.
"""

import os
import sys

import numpy as np

B, M, N, K = 256, 512, 512, 128
NCORES = 8
BPC = B // NCORES  # batches per core
MT = M // 128  # m-tiles per batch
OG = 4  # batches per output DMA group (4 batches -> 4 MiB per dma_start)

_cache = {}
LAST_RESULTS = None  # BassKernelResults of the most recent run (for profiling)


def _build(alpha: float):
    from contextlib import ExitStack

    import concourse.bass as bass
    import concourse.mybir as mybir
    import concourse.tile as tile
    from concourse import bacc

    nc = bacc.Bacc("TRN2", debug=False, enable_asserts=False, num_devices=NCORES)
    # a and b packed along the free dim so ONE dma feeds both matmul
    # operands (keeps the matmul's sync-wait count within the ISA limit).
    ab = nc.dram_tensor(
        "ab", [BPC, K, M + N], mybir.dt.bfloat16, kind="ExternalInput"
    )
    out = nc.dram_tensor("out", [BPC, M, N], mybir.dt.float32, kind="ExternalOutput")

    ap_ab = ab.ap()
    # DRAM out viewed with the partition dim innermost of the row index:
    # [BPC, (t p), n] -> [g, p, i, t, n] so one DMA writes OG whole batches
    # from an SBUF tile laid out [p, i, t, n].
    ap_o = out.ap().rearrange("(g i) (t p) n -> g p i t n", p=128, i=OG)

    with ExitStack() as ctx:
        tc = ctx.enter_context(tile.TileContext(nc))
        ab_pool = ctx.enter_context(tc.tile_pool(name="ab", bufs=1))
        ps_pool = ctx.enter_context(tc.tile_pool(name="ps", bufs=8, space="PSUM"))
        o_pool = ctx.enter_context(tc.tile_pool(name="o", bufs=2))

        # Whole input resident in SBUF (64KB/partition) via ONE SWDGE DMA
        # (gpsimd -> DMASW sem lanes). The 8 output DMAs then own all 8
        # DMAHW lanes with no lane reuse, keeping every DMACopy at <=1
        # sync wait (walrus limit).
        ab_sb = ab_pool.tile([K, BPC, M + N], mybir.dt.bfloat16, tag="ab")
        nc.gpsimd.dma_start(ab_sb[:], ap_ab.rearrange("i k f -> k i f"))

        for g in range(BPC // OG):
            o_sb = o_pool.tile([128, OG, MT, N], mybir.dt.float32, tag="o")
            for gi in range(OG):
                i = g * OG + gi
                for mt in range(MT):
                    ps = ps_pool.tile([128, N], mybir.dt.float32, tag="ps")
                    nc.tensor.matmul(
                        ps[:],
                        ab_sb[:, i, mt * 128 : (mt + 1) * 128],
                        ab_sb[:, i, M : M + N],
                        start=True,
                        stop=True,
                    )
                    # Epilogue on ScalarE only; the out-DMA is issued from
                    # ScalarE's own HWDGE queue so same-engine program order
                    # covers the data dependency cheaply.
                    nc.scalar.mul(o_sb[:, gi, mt, :], ps[:], float(alpha))
            nc.scalar.dma_start(ap_o[g], o_sb[:])
    nc.compile()
    return nc


def _get_nc(alpha: float):
    key = np.float32(alpha).tobytes()
    if key not in _cache:
        _cache[key] = _build(alpha)
    return _cache[key]


def kernel(a, b, alpha):
    import ml_dtypes

    from concourse.bass_utils import run_bass_kernel_spmd

    global LAST_RESULTS

    a = np.asarray(a)
    b = np.asarray(b)
    alpha_f = float(np.float32(np.asarray(alpha)))

    # Exact int8 -> bf16 conversion + transpose so K is the leading
    # (partition) dim on device: [B, M, K] -> [B, K, M]; a and b packed
    # side by side along the free dim.
    abT = np.empty((B, K, M + N), dtype=ml_dtypes.bfloat16)
    abT[:, :, :M] = np.swapaxes(a.astype(ml_dtypes.bfloat16), 1, 2)
    abT[:, :, M:] = np.swapaxes(b.astype(ml_dtypes.bfloat16), 1, 2)

    nc = _get_nc(alpha_f)
    in_maps = [
        {"ab": abT[c * BPC : (c + 1) * BPC]}
        for c in range(NCORES)
    ]
    res = run_bass_kernel_spmd(nc, in_maps, core_ids=list(range(NCORES)))
    LAST_RESULTS = res
    return np.concatenate([r["out"] for r in res.results], axis=0)


# revision 15
# speedup vs baseline: 1.1809x; 1.1809x over previous
"""Bass/Trainium2 kernel for batched int8 matmul with fp32 dequant epilogue.

Computes out[b, m, n] = alpha * sum_k a[b, m, k] * b[b, n, k] for
a, b int8 [256, 512, 128], out fp32 [256, 512, 512].

Strategy:
  - Shard the batch dim B=256 across 8 NeuronCores (32 batches/core).
  - int8 values convert EXACTLY to bf16 (8-bit significand covers +-256);
    products are ints <= 2^14 and the K=128 accumulation stays <= 2^21,
    exactly representable in the fp32 PSUM accumulator -> the bf16 matmul
    reproduces the int32-accumulated reference bit-exactly.
  - Host pre-transposes both operands to [B, K, M/N] so K lands on the
    SBUF partition dim (the PE contracts over partitions) with fully
    contiguous DMA rows.
  - K=128 means each [128m x 512n] output tile is a single matmul
    (no accumulation loop). alpha is folded into the PSUM->SBUF copy,
    alternating ScalarE/VectorE; fp32 out DMAs back to HBM.
"""

import os
import sys

import numpy as np

B, M, N, K = 256, 512, 512, 128
NCORES = 8
BPC = B // NCORES  # batches per core
MT = M // 128  # m-tiles per batch
OG = 2  # batches per output DMA group (2 batches -> 2 MiB per dma_start)
IG = 4  # batches per input DMA chunk (4 batches -> 1 MiB per dma_start)

_cache = {}
LAST_RESULTS = None  # BassKernelResults of the most recent run (for profiling)


def _build(alpha: float):
    from contextlib import ExitStack

    import concourse.bass as bass
    import concourse.mybir as mybir
    import concourse.tile as tile
    from concourse import bacc

    nc = bacc.Bacc("TRN2", debug=False, enable_asserts=False, num_devices=NCORES)
    # a and b packed along the free dim so ONE dma feeds both matmul
    # operands (keeps the matmul's sync-wait count within the ISA limit).
    ab = nc.dram_tensor(
        "ab", [BPC, K, M + N], mybir.dt.bfloat16, kind="ExternalInput"
    )
    out = nc.dram_tensor("out", [BPC, M, N], mybir.dt.float32, kind="ExternalOutput")

    ap_ab = ab.ap()
    # DRAM out viewed with the partition dim innermost of the row index:
    # [BPC, (t p), n] -> [g, p, i, t, n] so one DMA writes OG whole batches
    # from an SBUF tile laid out [p, i, t, n].
    ap_o = out.ap().rearrange("(g i) (t p) n -> g p i t n", p=128, i=OG)

    with ExitStack() as ctx:
        tc = ctx.enter_context(tile.TileContext(nc))
        ab_pool = ctx.enter_context(tc.tile_pool(name="ab", bufs=1))
        ps_pool = ctx.enter_context(tc.tile_pool(name="ps", bufs=8, space="PSUM"))
        o_pool = ctx.enter_context(tc.tile_pool(name="o", bufs=3))

        # Whole input resident in SBUF (64KB/partition), streamed in as
        # 1MiB chunks so the first matmuls start ~4us in instead of
        # waiting for the full 8MiB.
        ab_sb = ab_pool.tile([K, BPC, M + N], mybir.dt.bfloat16, tag="ab")
        for c0 in range(0, BPC, IG):
            nc.sync.dma_start(
                ab_sb[:, c0 : c0 + IG, :],
                ap_ab[c0 : c0 + IG].rearrange("i k f -> k i f"),
            )

        for g in range(BPC // OG):
            o_sb = o_pool.tile([128, OG, MT, N], mybir.dt.float32, tag="o")
            for gi in range(OG):
                i = g * OG + gi
                for mt in range(MT):
                    ps = ps_pool.tile([128, N], mybir.dt.float32, tag="ps")
                    nc.tensor.matmul(
                        ps[:],
                        ab_sb[:, i, mt * 128 : (mt + 1) * 128],
                        ab_sb[:, i, M : M + N],
                        start=True,
                        stop=True,
                    )
                    # Epilogue split across ScalarE and VectorE (each alone
                    # saturates; together they hide under the out-DMA stream).
                    dst = o_sb[:, gi, mt, :]
                    if (i * MT + mt) % 2 == 0:
                        nc.scalar.mul(dst, ps[:], float(alpha))
                    else:
                        nc.vector.tensor_scalar_mul(dst, ps[:], float(alpha))
            # Alternate output DMAs across the two HWDGE queues.
            if g % 2 == 0:
                nc.scalar.dma_start(ap_o[g], o_sb[:])
            else:
                nc.sync.dma_start(ap_o[g], o_sb[:])
    nc.compile()
    return nc


def _get_nc(alpha: float):
    key = np.float32(alpha).tobytes()
    if key not in _cache:
        _cache[key] = _build(alpha)
    return _cache[key]


def kernel(a, b, alpha):
    import ml_dtypes

    from concourse.bass_utils import run_bass_kernel_spmd

    global LAST_RESULTS

    a = np.asarray(a)
    b = np.asarray(b)
    alpha_f = float(np.float32(np.asarray(alpha)))

    # Exact int8 -> bf16 conversion + transpose so K is the leading
    # (partition) dim on device: [B, M, K] -> [B, K, M]; a and b packed
    # side by side along the free dim.
    abT = np.empty((B, K, M + N), dtype=ml_dtypes.bfloat16)
    abT[:, :, :M] = np.swapaxes(a.astype(ml_dtypes.bfloat16), 1, 2)
    abT[:, :, M:] = np.swapaxes(b.astype(ml_dtypes.bfloat16), 1, 2)

    nc = _get_nc(alpha_f)
    in_maps = [
        {"ab": abT[c * BPC : (c + 1) * BPC]}
        for c in range(NCORES)
    ]
    res = run_bass_kernel_spmd(nc, in_maps, core_ids=list(range(NCORES)))
    LAST_RESULTS = res
    return np.concatenate([r["out"] for r in res.results], axis=0)
